# revision 9
# baseline (speedup 1.0000x reference)
"""AttentiveMatch kernel for Trainium2 (8 NeuronCores, data-parallel over batch).

Reference math (per batch):
    pn = l2norm(p); qn = l2norm(q)
    w  = -(pn @ qn^T) / D          # [S,S]
    mv = (w @ q) / S               # [S,D]
    mn = l2norm(mv)
    out = -mean(pn * mn, -1)       # [S]

Device pipeline (scalars folded, sign flips cancel):
    G^T  = q @ p^T                       [S,S]   matmul 1 (PSUM, fp32)
    A^T  = diag(1/|q_j|) G^T             scale fused into PSUM->SBUF copy
    M^T  = q^T A                         [D,S]   matmul 2 (lhsT = q natural)
    dot_i = p_i . M_i = sum_j (1/|q_j|) (G^T)^2[j,i]   (matmul with rq weights)
    ss_i  = |M_i|^2  = sum_d (M^T)^2[d,i]              (matmul with ones)
    out_i = (1/D) dot_i / (|p_i| sqrt(ss_i))

Each core handles 8 batches; inputs shipped as bf16 in natural and
transposed layouts; all accumulation fp32.
"""

import sys

for _p in ("/opt/trn_rl_repo",):
    if _p not in sys.path:
        sys.path.append(_p)

import numpy as np
import ml_dtypes

import concourse.bacc as bacc
import concourse.mybir as mybir
import concourse.tile as tile
from concourse.bass_utils import run_bass_kernel_spmd

B, S, D = 64, 512, 768
NCORES = 8
BP = B // NCORES          # batches per core
ST = S // 128             # s tiles (4)
KT = D // 128             # d tiles (6)
F32 = mybir.dt.float32
BF16 = mybir.dt.bfloat16
AF = mybir.ActivationFunctionType
ALU = mybir.AluOpType

_NC = None


def _build():
    nc = bacc.Bacc("TRN2", target_bir_lowering=False, debug=False, num_devices=NCORES)
    pn_d = nc.dram_tensor("pn", [BP, 128, ST * D], BF16, kind="ExternalInput")
    qn_d = nc.dram_tensor("qn", [BP, 128, ST * D], BF16, kind="ExternalInput")
    pt_d = nc.dram_tensor("pt", [BP, 128, KT * S], BF16, kind="ExternalInput")
    qt_d = nc.dram_tensor("qt", [BP, 128, KT * S], BF16, kind="ExternalInput")
    out_d = nc.dram_tensor("out", [128, BP * ST], F32, kind="ExternalOutput")

    with tile.TileContext(nc) as tc:
        with (
            tc.tile_pool(name="cst", bufs=1) as cst,
            tc.tile_pool(name="inp", bufs=3) as inp,
            tc.tile_pool(name="ats", bufs=2) as ats,
            tc.tile_pool(name="gps", bufs=3, space="PSUM") as gps,
            tc.tile_pool(name="mps", bufs=3, space="PSUM") as mps,
            tc.tile_pool(name="rps", bufs=1, space="PSUM") as rps,
            tc.tile_pool(name="tps", bufs=1, space="PSUM") as tps,
            tc.tile_pool(name="scr", bufs=2) as scr,
            tc.tile_pool(name="st", bufs=2) as st,
            tc.tile_pool(name="res", bufs=1) as res,
        ):
            wd = res.tile([128, BP * ST], F32)
            ones16 = cst.tile([128, 1], BF16)
            nc.gpsimd.memset(ones16[:], 1.0)
            onef = cst.tile([128, 1], F32)
            nc.gpsimd.memset(onef[:], 1.0)

            for b in range(BP):
                qt_t = inp.tile([128, KT * S], BF16, tag="qt")
                nc.sync.dma_start(qt_t[:], qt_d[b])
                pt_t = inp.tile([128, KT * S], BF16, tag="pt")
                nc.sync.dma_start(pt_t[:], pt_d[b])
                q_t = inp.tile([128, ST * D], BF16, tag="q")
                nc.sync.dma_start(q_t[:], qn_d[b])
                p_t = inp.tile([128, ST * D], BF16, tag="p")
                nc.sync.dma_start(p_t[:], pn_d[b])

                # q row sum-of-squares via ACT Square+accumulate (needed for rq)
                ssq_q = st.tile([128, ST], F32, tag="ssq_q")
                for t in range(ST):
                    sl = slice(t * D, (t + 1) * D)
                    aq = scr.tile([128, D], BF16, tag="aq")
                    nc.scalar.activation(aq[:], q_t[:, sl], AF.Square,
                                         accum_out=ssq_q[:, t:t + 1])
                sq_q = st.tile([128, ST], F32, tag="sq_q")
                nc.scalar.activation(sq_q[:], ssq_q[:], AF.Sqrt)
                rq = st.tile([128, ST], F32, tag="rq")
                nc.vector.reciprocal(rq[:], sq_q[:])
                sqq16 = st.tile([128, ST], BF16, tag="sqq16")
                nc.vector.tensor_copy(sqq16[:], sq_q[:])

                rows = rps.tile([64, 512], F32, tag="rows")
                trn = tps.tile([128, 2 * ST], F32, tag="trn")

                # mm1: G^T[j,i] = sum_d q[j,d] p[i,d]; A^T = rq * G^T;
                # dot_i = sum_j sq_q[j] (A^T)^2[j,i]  (== sum_j rq_j G^2)
                at_tiles = []
                for j in range(ST):
                    g = gps.tile([128, S], F32, tag="g")
                    for k in range(KT):
                        nc.tensor.matmul(
                            g[:],
                            lhsT=qt_t[:, k * S + j * 128: k * S + (j + 1) * 128],
                            rhs=pt_t[:, k * S: (k + 1) * S],
                            start=(k == 0), stop=(k == KT - 1),
                        )
                    at = ats.tile([128, S], BF16, tag=f"at{j}")
                    nc.scalar.activation(at[:], g[:], AF.Copy, scale=rq[:, j:j + 1])
                    at_tiles.append(at)
                    h = scr.tile([128, S], BF16, tag="h")
                    nc.vector.tensor_mul(h[:], at[:], at[:])
                    nc.tensor.matmul(
                        rows[0:1, :], lhsT=sqq16[:, j:j + 1], rhs=h[:],
                        start=(j == 0), stop=(j == ST - 1),
                    )

                # mm2: M^T[d,i] = sum_j q[j,d] A^T[j,i]; ss_row += ones^T @ (M^T)^2
                for k in range(KT):
                    mt = mps.tile([128, S], F32, tag="mt")
                    for jt in range(ST):
                        nc.tensor.matmul(
                            mt[:],
                            lhsT=q_t[:, jt * D + k * 128: jt * D + (k + 1) * 128],
                            rhs=at_tiles[jt][:],
                            start=(jt == 0), stop=(jt == ST - 1),
                        )
                    ms = scr.tile([128, S], BF16, tag="ms")
                    nc.vector.tensor_copy(ms[:], mt[:])
                    s2 = scr.tile([128, S], BF16, tag="s2")
                    nc.vector.tensor_mul(s2[:], ms[:], ms[:])
                    nc.tensor.matmul(
                        rows[32:33, :], lhsT=ones16[:], rhs=s2[:],
                        start=(k == 0), stop=(k == KT - 1),
                    )

                # p row sum-of-squares (only needed for the finals -> late)
                ssq_p = st.tile([128, ST], F32, tag="ssq_p")
                for t in range(ST):
                    sl = slice(t * D, (t + 1) * D)
                    ap_ = scr.tile([128, D], BF16, tag="ap")
                    nc.scalar.activation(ap_[:], p_t[:, sl], AF.Square,
                                         accum_out=ssq_p[:, t:t + 1])
                sq_p = st.tile([128, ST], F32, tag="sq_p")
                nc.scalar.activation(sq_p[:], ssq_p[:], AF.Sqrt)
                rp = st.tile([128, ST], F32, tag="rp")
                nc.vector.reciprocal(rp[:], sq_p[:])

                # transpose the two [1,512] rows into [128, ST] columns
                rowsb = st.tile([64, 512], F32, tag="rowsb")
                nc.vector.tensor_copy(rowsb[:], rows[:])
                for c in range(ST):
                    nc.tensor.matmul(
                        trn[:, c:c + 1], lhsT=rowsb[0:1, c * 128:(c + 1) * 128],
                        rhs=onef[0:1, :], start=(c == 0), stop=False,
                    )
                for c in range(ST):
                    nc.tensor.matmul(
                        trn[:, ST + c: ST + c + 1],
                        lhsT=rowsb[32:33, c * 128:(c + 1) * 128],
                        rhs=onef[32:33, :], start=(c == 0), stop=(c == ST - 1),
                    )

                # wd = (1/D) * dot / (sq_p * sqrt(ss));  sqrt(D^2 ss) folds 1/D
                sd = st.tile([128, ST], F32, tag="sd")
                nc.scalar.activation(sd[:], trn[:, ST: 2 * ST], AF.Sqrt,
                                     scale=float(D) * float(D))
                rs = st.tile([128, ST], F32, tag="rs")
                nc.vector.reciprocal(rs[:], sd[:])
                w1 = st.tile([128, ST], F32, tag="w1")
                nc.vector.tensor_mul(w1[:], trn[:, 0:ST], rp[:])
                nc.vector.tensor_mul(wd[:, b * ST: (b + 1) * ST], w1[:], rs[:])

            nc.sync.dma_start(out_d[:], wd[:])
    nc.compile()
    return nc


def _get_nc():
    global _NC
    if _NC is None:
        _NC = _build()
    return _NC


def _prep_inputs(p, q):
    p = np.asarray(p, dtype=np.float32)
    q = np.asarray(q, dtype=np.float32)
    p16 = p.astype(ml_dtypes.bfloat16)
    q16 = q.astype(ml_dtypes.bfloat16)

    # natural: [core, b, part, t*D + d] with s = t*128 + part
    def nat(x):
        return np.ascontiguousarray(
            x.reshape(NCORES, BP, ST, 128, D).transpose(0, 1, 3, 2, 4)
        ).reshape(NCORES, BP, 128, ST * D)

    # transposed: [core, b, part, k*S + i] with d = k*128 + part
    def tr(x):
        return np.ascontiguousarray(
            x.reshape(NCORES, BP, S, KT, 128).transpose(0, 1, 4, 3, 2)
        ).reshape(NCORES, BP, 128, KT * S)

    pn, qn, pt, qt = nat(p16), nat(q16), tr(p16), tr(q16)
    return [
        {"pn": pn[c], "qn": qn[c], "pt": pt[c], "qt": qt[c]}
        for c in range(NCORES)
    ]


def _postprocess(results):
    o = np.stack([np.asarray(r["out"], dtype=np.float32) for r in results])
    # o[c, part, b*ST + t] is out for batch c*BP+b at i = t*128 + part
    o = o.reshape(NCORES, 128, BP, ST).transpose(0, 2, 3, 1).reshape(B, 1, S)
    return np.ascontiguousarray(o)


def _run(inputs, trace=False, **kw):
    nc = _get_nc()
    in_maps = _prep_inputs(inputs["p"], inputs["q"])
    res = run_bass_kernel_spmd(nc, in_maps, list(range(NCORES)), trace=trace, **kw)
    return _postprocess(res.results), res


def kernel(p, q):
    out, _ = _run({"p": p, "q": q})
    return out


# revision 17
# speedup vs baseline: 1.0596x; 1.0596x over previous
"""AttentiveMatch kernel for Trainium2 (8 NeuronCores, data-parallel over batch).

Reference math (per batch):
    pn = l2norm(p); qn = l2norm(q)
    w  = -(pn @ qn^T) / D          # [S,S]
    mv = (w @ q) / S               # [S,D]
    mn = l2norm(mv)
    out = -mean(pn * mn, -1)       # [S]

Device pipeline (scalars folded, sign flips cancel):
    G^T  = q @ p^T                       [S,S]   matmul 1 (PSUM, fp32)
    A^T  = diag(1/|q_j|) G^T             scale fused into PSUM->SBUF copy
    M^T  = q^T A                         [D,S]   matmul 2 (lhsT = q natural)
    dot_i = p_i . M_i = sum_j (1/|q_j|) (G^T)^2[j,i]   (matmul with rq weights)
    ss_i  = |M_i|^2  = sum_d (M^T)^2[d,i]              (matmul with ones)
    out_i = (1/D) dot_i / (|p_i| sqrt(ss_i))

Each core handles 8 batches; inputs shipped as bf16 in natural and
transposed layouts; all accumulation fp32.
"""

import sys

for _p in ("/opt/trn_rl_repo",):
    if _p not in sys.path:
        sys.path.append(_p)

import numpy as np
import ml_dtypes

import concourse.bacc as bacc
import concourse.mybir as mybir
import concourse.tile as tile
from concourse.bass_utils import run_bass_kernel_spmd

B, S, D = 64, 512, 768
NCORES = 8
BP = B // NCORES          # batches per core
ST = S // 128             # s tiles (4)
KT = D // 128             # d tiles (6)
F32 = mybir.dt.float32
F32R = mybir.dt.float32r
BF16 = mybir.dt.bfloat16
AF = mybir.ActivationFunctionType
ALU = mybir.AluOpType

_NC = None


def _build():
    nc = bacc.Bacc("TRN2", target_bir_lowering=False, debug=False, num_devices=NCORES)
    pn_d = nc.dram_tensor("pn", [BP, 128, ST * D], BF16, kind="ExternalInput")
    qn_d = nc.dram_tensor("qn", [BP, 128, ST * D], BF16, kind="ExternalInput")
    pt_d = nc.dram_tensor("pt", [BP, 128, KT * S], BF16, kind="ExternalInput")
    qt_d = nc.dram_tensor("qt", [BP, 128, KT * S], BF16, kind="ExternalInput")
    out_d = nc.dram_tensor("out", [128, BP * ST], F32, kind="ExternalOutput")

    with tile.TileContext(nc) as tc:
        with (
            tc.tile_pool(name="cst", bufs=1) as cst,
            tc.tile_pool(name="inp", bufs=3) as inp,
            tc.tile_pool(name="ats", bufs=2) as ats,
            tc.tile_pool(name="gps", bufs=3, space="PSUM") as gps,
            tc.tile_pool(name="mps", bufs=3, space="PSUM") as mps,
            tc.tile_pool(name="rps", bufs=1, space="PSUM") as rps,
            tc.tile_pool(name="tps", bufs=1, space="PSUM") as tps,
            tc.tile_pool(name="scr", bufs=2) as scr,
            tc.tile_pool(name="st", bufs=2) as st,
            tc.tile_pool(name="res", bufs=1) as res,
        ):
            wd = res.tile([128, BP * ST], F32)
            ones16 = cst.tile([128, 1], BF16)
            nc.gpsimd.memset(ones16[:], 1.0)
            onef = cst.tile([128, 1], F32)
            nc.gpsimd.memset(onef[:], 1.0)

            for b in range(BP):
                # qt/pt split into 3 chunks (2 k-tiles each) so mm1 can
                # start as soon as the first chunks land
                qt_c = []
                pt_c = []
                for c in range(3):
                    qc = inp.tile([128, 2 * S], BF16, tag=f"qt{c}")
                    nc.sync.dma_start(qc[:], qt_d[b, :, c * 2 * S:(c + 1) * 2 * S])
                    pc = inp.tile([128, 2 * S], BF16, tag=f"pt{c}")
                    nc.sync.dma_start(pc[:], pt_d[b, :, c * 2 * S:(c + 1) * 2 * S])
                    qt_c.append(qc)
                    pt_c.append(pc)
                q_t = inp.tile([128, ST * D], BF16, tag="q")
                nc.sync.dma_start(q_t[:], qn_d[b])
                p_t = inp.tile([128, ST * D], BF16, tag="p")
                nc.sync.dma_start(p_t[:], pn_d[b])

                # q row sum-of-squares via ACT Square+accumulate (needed for rq)
                ssq_q = st.tile([128, ST], F32, tag="ssq_q")
                for t in range(ST):
                    sl = slice(t * D, (t + 1) * D)
                    aq = scr.tile([128, D], BF16, tag="aq")
                    nc.scalar.activation(aq[:], q_t[:, sl], AF.Square,
                                         accum_out=ssq_q[:, t:t + 1])
                sq_q = st.tile([128, ST], F32, tag="sq_q")
                nc.scalar.activation(sq_q[:], ssq_q[:], AF.Sqrt)
                rq = st.tile([128, ST], F32, tag="rq")
                nc.vector.reciprocal(rq[:], sq_q[:])
                sqq16 = st.tile([128, ST], BF16, tag="sqq16")
                nc.vector.tensor_copy(sqq16[:], sq_q[:])

                rows = rps.tile([64, 512], F32, tag="rows")
                trn = tps.tile([128, 2 * ST], F32, tag="trn")

                # mm1: G^T[j,i] = sum_d q[j,d] p[i,d]; A^T = rq * G^T;
                # dot_i = sum_j sq_q[j] (A^T)^2[j,i]  (== sum_j rq_j G^2)
                at_tiles = []
                for j in range(ST):
                    g = gps.tile([128, S], F32, tag="g")
                    for k in range(KT):
                        kc, ko = divmod(k, 2)
                        nc.tensor.matmul(
                            g[:],
                            lhsT=qt_c[kc][:, ko * S + j * 128: ko * S + (j + 1) * 128],
                            rhs=pt_c[kc][:, ko * S: (ko + 1) * S],
                            start=(k == 0), stop=(k == KT - 1),
                        )
                    at = ats.tile([128, S], BF16, tag=f"at{j}")
                    nc.scalar.activation(at[:], g[:], AF.Copy, scale=rq[:, j:j + 1])
                    at_tiles.append(at)
                    h = scr.tile([128, S], BF16, tag="h")
                    nc.vector.tensor_mul(h[:], at[:], at[:])
                    nc.tensor.matmul(
                        rows[0:1, :], lhsT=sqq16[:, j:j + 1], rhs=h[:],
                        start=(j == 0), stop=(j == ST - 1),
                    )

                # mm2: M^T[d,i] = sum_j q[j,d] A^T[j,i]; ss_row += ones^T @ (M^T)^2
                # (s2 tiles pre-summed in pairs on DVE -> 3 ones-matmuls)
                s2_pair = []
                for k in range(KT):
                    mt = mps.tile([128, S], F32, tag="mt")
                    for jt in range(ST):
                        nc.tensor.matmul(
                            mt[:],
                            lhsT=q_t[:, jt * D + k * 128: jt * D + (k + 1) * 128],
                            rhs=at_tiles[jt][:],
                            start=(jt == 0), stop=(jt == ST - 1),
                        )
                    ms = scr.tile([128, S], BF16, tag="ms")
                    nc.vector.tensor_copy(ms[:], mt[:])
                    s2 = scr.tile([128, S], BF16, tag=f"s2{k % 2}")
                    nc.vector.tensor_mul(s2[:], ms[:], ms[:])
                    s2_pair.append(s2)
                    if k % 2 == 1:
                        s2s = scr.tile([128, S], BF16, tag="s2s")
                        nc.vector.tensor_add(s2s[:], s2_pair[0][:], s2_pair[1][:])
                        s2_pair = []
                        nc.tensor.matmul(
                            rows[32:33, :], lhsT=ones16[:], rhs=s2s[:],
                            start=(k == 1), stop=(k == KT - 1),
                        )

                # p row sum-of-squares (only needed for the finals -> late)
                ssq_p = st.tile([128, ST], F32, tag="ssq_p")
                for t in range(ST):
                    sl = slice(t * D, (t + 1) * D)
                    ap_ = scr.tile([128, D], BF16, tag="ap")
                    nc.scalar.activation(ap_[:], p_t[:, sl], AF.Square,
                                         accum_out=ssq_p[:, t:t + 1])
                sq_p = st.tile([128, ST], F32, tag="sq_p")
                nc.scalar.activation(sq_p[:], ssq_p[:], AF.Sqrt)
                rp = st.tile([128, ST], F32, tag="rp")
                nc.vector.reciprocal(rp[:], sq_p[:])

                # transpose the two [1,512] rows into [128, ST] columns
                rowsb = st.tile([64, 512], F32, tag="rowsb")
                nc.vector.tensor_copy(rowsb[:], rows[:])
                for c in range(ST):
                    nc.tensor.matmul(
                        trn[:, c:c + 1],
                        lhsT=rowsb[0:1, c * 128:(c + 1) * 128],
                        rhs=onef[0:1, :], start=(c == 0), stop=False,
                    )
                for c in range(ST):
                    nc.tensor.matmul(
                        trn[:, ST + c: ST + c + 1],
                        lhsT=rowsb[32:33, c * 128:(c + 1) * 128],
                        rhs=onef[32:33, :], start=(c == 0), stop=(c == ST - 1),
                    )

                # wd = (1/D) * dot / (sq_p * sqrt(ss));  sqrt(D^2 ss) folds 1/D
                sd = st.tile([128, ST], F32, tag="sd")
                nc.scalar.activation(sd[:], trn[:, ST: 2 * ST], AF.Sqrt,
                                     scale=float(D) * float(D))
                rs = st.tile([128, ST], F32, tag="rs")
                nc.vector.reciprocal(rs[:], sd[:])
                w1 = st.tile([128, ST], F32, tag="w1")
                nc.vector.tensor_mul(w1[:], trn[:, 0:ST], rp[:])
                nc.vector.tensor_mul(wd[:, b * ST: (b + 1) * ST], w1[:], rs[:])

            nc.sync.dma_start(out_d[:], wd[:])
    nc.compile()
    return nc


def _get_nc():
    global _NC
    if _NC is None:
        _NC = _build()
    return _NC


def _prep_inputs(p, q):
    p = np.asarray(p, dtype=np.float32)
    q = np.asarray(q, dtype=np.float32)
    p16 = p.astype(ml_dtypes.bfloat16)
    q16 = q.astype(ml_dtypes.bfloat16)

    # natural: [core, b, part, t*D + d] with s = t*128 + part
    def nat(x):
        return np.ascontiguousarray(
            x.reshape(NCORES, BP, ST, 128, D).transpose(0, 1, 3, 2, 4)
        ).reshape(NCORES, BP, 128, ST * D)

    # transposed: [core, b, part, k*S + i] with d = k*128 + part
    def tr(x):
        return np.ascontiguousarray(
            x.reshape(NCORES, BP, S, KT, 128).transpose(0, 1, 4, 3, 2)
        ).reshape(NCORES, BP, 128, KT * S)

    pn, qn, pt, qt = nat(p16), nat(q16), tr(p16), tr(q16)
    return [
        {"pn": pn[c], "qn": qn[c], "pt": pt[c], "qt": qt[c]}
        for c in range(NCORES)
    ]


def _postprocess(results):
    o = np.stack([np.asarray(r["out"], dtype=np.float32) for r in results])
    # o[c, part, b*ST + t] is out for batch c*BP+b at i = t*128 + part
    o = o.reshape(NCORES, 128, BP, ST).transpose(0, 2, 3, 1).reshape(B, 1, S)
    return np.ascontiguousarray(o)


def _run(inputs, trace=False, **kw):
    nc = _get_nc()
    in_maps = _prep_inputs(inputs["p"], inputs["q"])
    res = run_bass_kernel_spmd(nc, in_maps, list(range(NCORES)), trace=trace, **kw)
    return _postprocess(res.results), res


def kernel(p, q):
    out, _ = _run({"p": p, "q": q})
    return out
